# revision 1
# baseline (speedup 1.0000x reference)
"""Trainium2 Bass kernel for a single-query attention layer.

Reference computation (per batch b):
    q      = ht[b] @ W                      # (1, H)
    scores = q . h_0_t[b, t, :] over H      # (T,)
    alpha  = softmax(scores)                # (T,)
    ct[b]  = sum_t alpha[t] * h_0_t[b, t]   # (1, H)

Sharding: data-parallel over batch across 8 NeuronCores (8 batches per
core); the (H, H) weight is replicated.  No collectives.

Per-core dataflow (memory-bound; the 64 MiB h_0_t shard is read from
HBM exactly once):
  - stream h_0_t[b] as one 8 MiB whole-batch DMA into SBUF, natural [T-on-partitions,
    H-on-free] layout (the only DMA-efficient one for this DRAM layout);
    big chunks amortize SWDGE per-DMA Q7 descriptor emission;
    chunk DMAs ALTERNATE between the two HWDGE rings (SP=qSPDynamicHW,
    ACT=qActDynamicHW) so per-DMA descriptor-generation/completion
    bookkeeping on one ring hides under the other ring's transfer
    (measured -17% vs all-chunks-on-SP in same-round paired A/B)
  - chunks land as fp16: SWDGE casts f32->fp16 inside the SDMA datapath
    (HBM reads unchanged, SBUF writes halve, DVE scores run at 2x,
    PE consumes fp16 at full rate; rel err 3.8e-3 vs 2e-2 tolerance)
  - scores: one fused multiply+reduce (tensor_tensor_reduce) per
    128-timestep tile on VectorE, against a partition-replicated q
  - softmax: free-axis max (DVE) -> partition max via PE transpose +
    free-axis max -> exp with fused bias and fused row-sum (ScalarE) ->
    partition sum via a ones-matmul (PE)
  - weighted sum: 64 TensorE matmuls per batch (contraction over T =
    partition axis, which the natural layout supports directly), fp32r
    at full PE rate, accumulated in PSUM
  - scale by 1/denominator during the PSUM->SBUF copy, DMA the row out

Batches are pipelined: the chunk pool holds 2 batches so batch b+1's
DMA+scores overlap batch b's softmax+weighted-sum.
"""

import sys

import numpy as np

_BASS_ROOT = "/opt/trn_rl_repo"
if _BASS_ROOT not in sys.path:
    sys.path.insert(0, _BASS_ROOT)

import concourse.bass as bass  # noqa: E402
from concourse import mybir  # noqa: E402
from concourse.bass_utils import run_bass_kernel_spmd  # noqa: E402
from concourse.tile import TileContext  # noqa: E402

B, T, H = 64, 8192, 256
N_CORES = 8
B_LOC = B // N_CORES  # batches per core
P = 128               # SBUF partitions
F32 = mybir.dt.float32
F32R = mybir.dt.float32r
I32 = mybir.dt.int32


def build_nc(b_loc=B_LOC, t=T, chunk_k=64, h_bufs=4, reps=1,
             dual_ring=True, out_ring="sync", tri_ring=False,
             score_stride=1, bf16_chunks=True):
    """Build the per-core Bass graph.

    chunk_k: T-tiles (of 128 timesteps) per DMA chunk.
    h_bufs:  chunk-pool slots (h_bufs * chunk_k * 128 timesteps resident).
    reps:    unrolled repetitions of the whole batch loop (benchmarking
             aid — wall(reps=3) - wall(reps=1) = 2x the kernel time,
             cancelling dispatch overhead).
    dual_ring: alternate chunk DMAs between the SP and ACT HWDGE rings.
    out_ring: 'sync' or 'scalar' — ring for the per-batch output DMA.
    """
    tpb = t // P              # T-tiles per batch
    n_chunks = tpb // chunk_k
    hc = H // P               # contraction chunks for the q matmul

    from concourse.bacc import Bacc

    nc = Bacc()
    h_d = nc.declare_dram_parameter("h_0_t", [b_loc, t, H], F32, isOutput=False)
    ht_d = nc.declare_dram_parameter("ht", [b_loc, 1, H], F32, isOutput=False)
    w_d = nc.declare_dram_parameter("weight", [H, H], F32, isOutput=False)
    out_d = nc.declare_dram_parameter("out", [b_loc, 1, H], F32, isOutput=True)

    with TileContext(nc) as tc:
        with (
            tc.tile_pool(name="const", bufs=1) as const_pool,
            tc.tile_pool(name="hbuf", bufs=h_bufs) as h_pool,
            tc.tile_pool(name="stats", bufs=2) as stats_pool,
            tc.tile_pool(name="scr", bufs=2) as scr_pool,
            tc.tile_pool(name="ctout", bufs=2) as out_pool,
            tc.tile_pool(name="ps", bufs=1, space="PSUM") as psum_pool,
            tc.tile_pool(name="qdram", bufs=1, space="DRAM") as dram_pool,
        ):
            # ---- constants ----
            ones_col = const_pool.tile([P, 1], F32, name="ones_col")
            nc.vector.memset(ones_col, 1.0)
            neg_ones_row = const_pool.tile([1, P], F32, name="neg_ones_row")
            nc.vector.memset(neg_ones_row, -1.0)
            ident_i = const_pool.tile([P, P], I32, name="ident_i")
            nc.gpsimd.iota(ident_i, pattern=[[-1, P]], base=0, channel_multiplier=1)
            ident = const_pool.tile([P, P], F32, name="ident")
            nc.vector.tensor_scalar(
                ident, ident_i, 0, None, op0=mybir.AluOpType.is_equal
            )

            # ---- q = ht @ W for all local batches (one-time setup) ----
            w_sb = const_pool.tile([P, hc, H], F32, name="w_sb")
            nc.sync.dma_start(
                out=w_sb, in_=w_d[:].rearrange("(c p) k -> p c k", p=P)
            )
            htT = const_pool.tile([P, hc, b_loc], F32, name="htT")
            for c in range(hc):
                nc.gpsimd.dma_start(
                    out=htT[:, c, :],
                    in_=ht_d[:, 0, c * P : (c + 1) * P].rearrange("b p -> p b"),
                )
            # dummy self-matmul absorbs the htT DMA wait so the q matmul
            # carries a single sync wait (PE LDWEIGHTS allows only one)
            dmy_ps = psum_pool.tile(
                [b_loc, b_loc], F32, name="dmy_ps", tag="dmy", bufs=2
            )
            nc.tensor.matmul(
                dmy_ps, lhsT=htT[:, 0, :], rhs=htT[:, 0, :], start=True, stop=True
            )
            q_ps = psum_pool.tile([b_loc, H], F32, name="q_ps", tag="qps")
            for c in range(hc):
                nc.tensor.matmul(
                    q_ps, lhsT=htT[:, c, :], rhs=w_sb[:, c, :],
                    start=(c == 0), stop=(c == hc - 1),
                )
            q_sb = const_pool.tile([b_loc, H], F32, name="q_sb")
            nc.vector.tensor_copy(q_sb, q_ps)
            # replicate each batch's q across all 128 partitions (DRAM bounce)
            q_dram = dram_pool.tile([b_loc, H], F32, name="q_dram")
            nc.sync.dma_start(out=q_dram, in_=q_sb)
            q_rep = const_pool.tile([P, b_loc, H], F32, name="q_rep")
            q_bcast_src = bass.AP(
                tensor=q_dram.tensor, offset=q_dram.offset,
                ap=[[0, P], [H, b_loc], [1, H]],
            )
            nc.sync.dma_start(out=q_rep, in_=q_bcast_src)
            if bf16_chunks:
                BF16 = mybir.dt.float16
                q_rep_bf = const_pool.tile(
                    [P, b_loc, H], BF16, name="q_rep_bf"
                )
                nc.vector.tensor_copy(q_rep_bf, q_rep)

            # ---- batch loop ----
            for b in [bb for _ in range(reps) for bb in range(b_loc)]:
                s_all = stats_pool.tile([P, tpb], F32, name="s_all", tag="s_all")
                chunks = []
                for c in range(n_chunks):
                    # declared fp32r so TensorE can consume it at full rate;
                    # the DVE scores path reads the same bits as fp32.
                    # Blocked T layout: partition p holds chunk_k CONSECUTIVE
                    # timesteps (contiguous chunk_k*1KB DRAM per partition ->
                    # large DMA descriptors). softmax + weighted sum are
                    # permutation-invariant over T, so the order change is
                    # harmless.
                    src = h_d[b, c * chunk_k * P : (c + 1) * chunk_k * P, :]
                    if bf16_chunks:
                        # SWDGE casts f32->fp16 inside the SDMA datapath
                        # (free); HBM read side is unchanged, SBUF write
                        # side halves, DVE scores run at 2x on 16-bit and
                        # PE consumes bf16 at full rate.
                        hch = h_pool.tile(
                            [P, chunk_k, H], mybir.dt.float16,
                            name="hch", tag="hch",
                        )
                        nc.gpsimd.dma_start(
                            out=hch,
                            in_=src.rearrange("(p k) j -> p k j", k=chunk_k),
                        )
                    else:
                        hch = h_pool.tile(
                            [P, chunk_k, H], F32R, name="hch", tag="hch"
                        )
                        if tri_ring:
                            dma_eng = [nc.sync, nc.scalar, nc.gpsimd][c % 3]
                        else:
                            dma_eng = (
                                nc.scalar if (dual_ring and c % 2) else nc.sync
                            )
                        dma_eng.dma_start(
                            out=hch,
                            in_=src.rearrange(
                                "(p k) j -> p k j", k=chunk_k
                            ).bitcast(F32R),
                        )
                    chunks.append(hch)
                    for k in range(0, chunk_k, score_stride):
                        gk = c * chunk_k + k
                        if bf16_chunks:
                            vscr = scr_pool.tile(
                                [P, H], mybir.dt.float16, name="vscr",
                                tag="vscr",
                            )
                            nc.vector.scalar_tensor_tensor(
                                out=vscr, in0=hch[:, k, :],
                                scalar=1.0, in1=q_rep_bf[:, b, :],
                                op0=mybir.AluOpType.mult,
                                op1=mybir.AluOpType.mult,
                                accum_out=s_all[:, gk : gk + 1],
                            )
                        else:
                            vscr = scr_pool.tile(
                                [P, H], F32, name="vscr", tag="vscr"
                            )
                            nc.vector.scalar_tensor_tensor(
                                out=vscr, in0=hch[:, k, :].bitcast(F32),
                                scalar=1.0, in1=q_rep[:, b, :],
                                op0=mybir.AluOpType.mult,
                                op1=mybir.AluOpType.mult,
                                accum_out=s_all[:, gk : gk + 1],
                            )

                # ---- softmax statistics ----
                m_col = stats_pool.tile([P, 1], F32, name="m_col", tag="m_col")
                nc.vector.reduce_max(m_col, s_all, axis=mybir.AxisListType.X)
                mT_ps = psum_pool.tile([1, P], F32, name="mT_ps", tag="mT")
                nc.tensor.transpose(mT_ps, m_col, ident)
                m_sb = stats_pool.tile([1, 1], F32, name="m_sb", tag="m_sb")
                nc.vector.reduce_max(m_sb, mT_ps, axis=mybir.AxisListType.X)
                # broadcast -max to all partitions via a C=1 matmul
                negm_ps = psum_pool.tile([P, 1], F32, name="negm_ps", tag="negm")
                nc.tensor.matmul(
                    negm_ps, lhsT=neg_ones_row, rhs=m_sb, start=True, stop=True
                )
                negm_sb = stats_pool.tile([P, 1], F32, name="negm_sb", tag="negm_sb")
                nc.vector.tensor_copy(negm_sb, negm_ps)
                p_dt = mybir.dt.float16 if bf16_chunks else F32R
                p_all = stats_pool.tile([P, tpb], p_dt, name="p_all", tag="p_all")
                l_col = stats_pool.tile([P, 1], F32, name="l_col", tag="l_col")
                nc.scalar.activation(
                    out=p_all, in_=s_all, func=mybir.ActivationFunctionType.Exp,
                    bias=negm_sb, scale=1.0, accum_out=l_col,
                )
                l_ps = psum_pool.tile([1, 1], F32, name="l_ps", tag="l")
                nc.tensor.matmul(
                    l_ps, lhsT=l_col, rhs=ones_col, start=True, stop=True
                )
                inv_l = stats_pool.tile([1, 1], F32, name="inv_l", tag="inv_l")
                nc.vector.reciprocal(inv_l, l_ps)

                # ---- weighted sum over T on TensorE ----
                ct_ps = psum_pool.tile([1, H], F32, name="ct_ps", tag="ct", bufs=2)
                for c in range(n_chunks):
                    for k in range(chunk_k):
                        gk = c * chunk_k + k
                        nc.tensor.matmul(
                            ct_ps, lhsT=p_all[:, gk : gk + 1],
                            rhs=chunks[c][:, k, :],
                            start=(gk == 0), stop=(gk == tpb - 1),
                        )
                ct_sb = out_pool.tile([1, H], F32, name="ct_sb", tag="ct_sb")
                nc.vector.tensor_scalar_mul(ct_sb, ct_ps, inv_l[0:1, 0:1])
                out_eng = nc.scalar if out_ring == "scalar" else nc.sync
                out_eng.dma_start(out=out_d[b, :, :], in_=ct_sb)

    # Bacc.finalize() runs the lowering passes raw Bass lacks: matmul-wait
    # relocation, event-semaphore wait splitting (HW allows 1 wait/inst),
    # GPSIMD library loads, ACT table loads, and extended-ISA codegen.
    if not nc.is_finalized():
        nc.finalize()
    return nc


_nc_cache = None


def _get_nc():
    global _nc_cache
    if _nc_cache is None:
        _nc_cache = build_nc()
    return _nc_cache


def _run(inputs, trace=False, **kw):
    nc = _get_nc()
    ht = np.ascontiguousarray(np.asarray(inputs["ht"], dtype=np.float32))
    h0 = np.asarray(inputs["h_0_t"], dtype=np.float32)
    w = np.ascontiguousarray(np.asarray(inputs["weight"], dtype=np.float32))
    in_maps = []
    for i in range(N_CORES):
        sl = slice(i * B_LOC, (i + 1) * B_LOC)
        in_maps.append(
            {
                "h_0_t": np.ascontiguousarray(h0[sl]),
                "ht": np.ascontiguousarray(ht[sl]),
                "weight": w,
            }
        )
    res = run_bass_kernel_spmd(
        nc, in_maps, core_ids=list(range(N_CORES)), trace=trace, **kw
    )
    out = np.concatenate([r["out"] for r in res.results], axis=0)
    return out, res


def kernel(**inputs):
    out, _ = _run(inputs)
    return out


# ---------------------------------------------------------------------------
# Timing helper (used by test.py only; not part of the grading contract).
# Rebuilds the shard_map executable once so repeat calls reuse one compiled
# NEFF with device-resident inputs, then reports min wall-clock.
# ---------------------------------------------------------------------------


_nc_rep_cache = {}


def _get_exec(inputs, reps=1):
    """Build (once) and return a zero-arg callable running the reps-unrolled
    kernel on all 8 cores with device-resident inputs."""
    import jax
    from jax.experimental.shard_map import shard_map
    from jax.sharding import Mesh, NamedSharding, PartitionSpec

    from concourse import bass2jax

    if reps == 1:
        nc = _get_nc()
    else:
        if reps not in _nc_rep_cache:
            _nc_rep_cache[reps] = build_nc(reps=reps)
        nc = _nc_rep_cache[reps]
    bass2jax.install_neuronx_cc_hook()

    partition_name = (
        nc.partition_id_tensor.name if nc.partition_id_tensor else None
    )
    in_names, out_names, out_avals, zero_outs = [], [], [], []
    for alloc in nc.m.functions[0].allocations:
        if not isinstance(alloc, mybir.MemoryLocationSet):
            continue
        name = alloc.memorylocations[0].name
        if alloc.kind == "ExternalInput":
            if name != partition_name:
                in_names.append(name)
        elif alloc.kind == "ExternalOutput":
            out_names.append(name)
            shape = tuple(alloc.tensor_shape)
            dtype = mybir.dt.np(alloc.dtype)
            out_avals.append(jax.core.ShapedArray(shape, dtype))
            zero_outs.append(np.zeros(shape, dtype))
    n_params = len(in_names)
    n_outs = len(out_avals)
    all_names = list(in_names) + out_names
    if partition_name is not None:
        all_names.append(partition_name)

    def _body(*args):
        operands = list(args)
        if partition_name is not None:
            operands.append(bass2jax.partition_id_tensor())
        outs = bass2jax._bass_exec_p.bind(
            *operands,
            out_avals=tuple(out_avals),
            in_names=tuple(all_names),
            out_names=tuple(out_names),
            lowering_input_output_aliases=(),
            sim_require_finite=True,
            sim_require_nnan=True,
            nc=nc,
        )
        return tuple(outs)

    devices = jax.devices()[:N_CORES]
    mesh = Mesh(np.asarray(devices), ("core",))
    in_specs = (PartitionSpec("core"),) * (n_params + n_outs)
    out_specs = (PartitionSpec("core"),) * n_outs
    sharded = jax.jit(
        shard_map(
            _body, mesh=mesh, in_specs=in_specs, out_specs=out_specs,
            check_rep=False,
        ),
        keep_unused=True,
    )

    ht = np.ascontiguousarray(np.asarray(inputs["ht"], dtype=np.float32))
    h0 = np.ascontiguousarray(np.asarray(inputs["h_0_t"], dtype=np.float32))
    w = np.asarray(inputs["weight"], dtype=np.float32)
    per_core = {
        "h_0_t": h0,
        "ht": ht,
        "weight": np.concatenate([w[None]] * N_CORES, axis=0).reshape(
            N_CORES * w.shape[0], w.shape[1]
        ),
    }
    sh = NamedSharding(mesh, PartitionSpec("core"))
    xs = [jax.device_put(per_core[name], sh) for name in in_names]
    zs = [
        jax.device_put(
            np.zeros((N_CORES * z.shape[0], *z.shape[1:]), z.dtype), sh
        )
        for z in zero_outs
    ]

    def call():
        jax.block_until_ready(sharded(*xs, *zs))

    call()  # warm up (includes compile)
    return call


def time_kernel_pair(inputs, iters=60, reps_hi=3, reps_lo=1):
    """Interleaved slope timing: min(wall_hi) - min(wall_lo) over paired
    adjacent samples cancels axon dispatch overhead and its drift.
    Returns one kernel execution time in ns."""
    import time

    lo = _get_exec(inputs, reps=reps_lo)
    hi = _get_exec(inputs, reps=reps_hi)
    t_lo, t_hi = [], []
    for _ in range(iters):
        t0 = time.perf_counter()
        lo()
        t1 = time.perf_counter()
        hi()
        t2 = time.perf_counter()
        t_lo.append(t1 - t0)
        t_hi.append(t2 - t1)
    ns = (min(t_hi) - min(t_lo)) / (reps_hi - reps_lo) * 1e9
    return ns, min(t_lo) * 1e9, min(t_hi) * 1e9



# revision 20
# speedup vs baseline: 1.0733x; 1.0733x over previous
"""Trainium2 Bass kernel for a single-query attention layer.

Reference computation (per batch b):
    q      = ht[b] @ W                      # (1, H)
    scores = q . h_0_t[b, t, :] over H      # (T,)
    alpha  = softmax(scores)                # (T,)
    ct[b]  = sum_t alpha[t] * h_0_t[b, t]   # (1, H)

Sharding: data-parallel over batch across 8 NeuronCores (8 batches per
core); the (H, H) weight is replicated.  No collectives.

Per-core dataflow (memory-bound; the 64 MiB h_0_t shard is read from
HBM exactly once):
  - stream h_0_t[b] as one 8 MiB whole-batch DMA into SBUF, natural [T-on-partitions,
    H-on-free] layout (the only DMA-efficient one for this DRAM layout);
    big chunks amortize SWDGE per-DMA Q7 descriptor emission;
    chunk DMAs ALTERNATE between the two HWDGE rings (SP=qSPDynamicHW,
    ACT=qActDynamicHW) so per-DMA descriptor-generation/completion
    bookkeeping on one ring hides under the other ring's transfer
    (measured -17% vs all-chunks-on-SP in same-round paired A/B)
  - chunks land as fp16: SWDGE casts f32->fp16 inside the SDMA datapath
    (HBM reads unchanged, SBUF writes halve, DVE scores run at 2x,
    PE consumes fp16 at full rate; rel err 3.8e-3 vs 2e-2 tolerance)
  - scores: one fused multiply+reduce (tensor_tensor_reduce) per
    128-timestep tile on VectorE, against a partition-replicated q
  - softmax: free-axis max (DVE) -> partition max via PE transpose +
    free-axis max -> exp with fused bias and fused row-sum (ScalarE) ->
    partition sum via a ones-matmul (PE)
  - weighted sum: 64 TensorE matmuls per batch (contraction over T =
    partition axis, which the natural layout supports directly), fp16
    at full PE rate, accumulated in PSUM
  - scale by 1/denominator during the PSUM->SBUF copy into a staging
    row; all 8 ct rows leave in ONE 8 KB DMA per pass (out_batch) --
    eight separate 1 KB output DMAs measurably steal DMA-ring
    arbitration slots from the chunk stream (~2.8 us/pass recovered
    under HBM contention, the entire kernel-vs-pure-DMA gap)

Batches are pipelined: the chunk pool holds 5 batches (h_bufs=5) so
several batches' DMAs are queued ahead of compute; the deeper queue
also wins slightly more HBM arbitration share when the stack is
contended by co-tenants (-0.6 us/pass).

Rejected by paired A/B (same-round interleaved hi-reps bursts, median
paired delta over 60+ rounds): half-fp16/half-f32r dual-queue chunks
(+5.6 us/pass in-kernel), 16 MiB two-batch chunks (+2.2 us/pass as pure
DMA), dual/tri-ring f32 chunk placement (neutral to +1.6).  The DMA
stream itself sits at the pairwise-stack HBM floor (~358 GB/s/core,
712 GB/s/stack measured = 99.5% of the 716 GB/s spec), so only the
few microseconds of non-DMA overhead were recoverable.
"""

import sys

import numpy as np

_BASS_ROOT = "/opt/trn_rl_repo"
if _BASS_ROOT not in sys.path:
    sys.path.insert(0, _BASS_ROOT)

import concourse.bass as bass  # noqa: E402
from concourse import mybir  # noqa: E402
from concourse.bass_utils import run_bass_kernel_spmd  # noqa: E402
from concourse.tile import TileContext  # noqa: E402

B, T, H = 64, 8192, 256
N_CORES = 8
B_LOC = B // N_CORES  # batches per core
P = 128               # SBUF partitions
F32 = mybir.dt.float32
F32R = mybir.dt.float32r
I32 = mybir.dt.int32


def build_nc_pair(b_loc=B_LOC, t=T, reps=1, h_bufs=2, split_dma=False):
    """Pair-chunk variant: one 16 MiB SWDGE fp16-cast DMA covers TWO
    batches (batch 2c on partitions 0-63, batch 2c+1 on 64-127), halving
    the number of DMA-queue entries vs build_nc.  Per-batch engine costs
    are kept equal to build_nc:

      - scores: one DVE inst per timestep-column across all 128 partitions,
        against q_pair (q[2c] replicated on the top half, q[2c+1] on the
        bottom half)
      - softmax max/sum are computed per batch-half via block-mask matmuls
        (a [2,P] neg-blockmask broadcasts the two maxima back to their
        partition halves; a [P,2] blockmask row-sums l per half)
      - weighted sum: one matmul per column with M=2 (lhsT = [P,2] with
        the top/bottom halves of p masked), accumulating ct for both
        batches into a [2,H] PSUM tile
    """
    tpb2 = 2 * (t // P)       # score columns per pair
    n_pairs = b_loc // 2
    hc = H // P

    from concourse.bacc import Bacc

    nc = Bacc()
    h_d = nc.declare_dram_parameter("h_0_t", [b_loc, t, H], F32, isOutput=False)
    ht_d = nc.declare_dram_parameter("ht", [b_loc, 1, H], F32, isOutput=False)
    w_d = nc.declare_dram_parameter("weight", [H, H], F32, isOutput=False)
    out_d = nc.declare_dram_parameter("out", [b_loc, 1, H], F32, isOutput=True)
    h_flat = h_d[:].rearrange("b t j -> (b t) j")
    BF16 = mybir.dt.float16

    with TileContext(nc) as tc:
        with (
            tc.tile_pool(name="const", bufs=1) as const_pool,
            tc.tile_pool(name="hbuf", bufs=h_bufs) as h_pool,
            tc.tile_pool(name="stats", bufs=2) as stats_pool,
            tc.tile_pool(name="scr", bufs=2) as scr_pool,
            tc.tile_pool(name="ctout", bufs=2) as out_pool,
            tc.tile_pool(name="ps", bufs=1, space="PSUM") as psum_pool,
            tc.tile_pool(name="qdram", bufs=1, space="DRAM") as dram_pool,
        ):
            # ---- constants ----
            ones_col = const_pool.tile([P, 1], F32, name="ones_col")
            nc.vector.memset(ones_col, 1.0)
            neg_ones_row = const_pool.tile([1, P], F32, name="neg_ones_row")
            nc.vector.memset(neg_ones_row, -1.0)
            ident_i = const_pool.tile([P, P], I32, name="ident_i")
            nc.gpsimd.iota(ident_i, pattern=[[-1, P]], base=0, channel_multiplier=1)
            ident = const_pool.tile([P, P], F32, name="ident")
            nc.vector.tensor_scalar(
                ident, ident_i, 0, None, op0=mybir.AluOpType.is_equal
            )
            # [2,P] neg-blockmask: row 0 = -1 on cols 0:64, row 1 = -1 on 64:128
            negblk = const_pool.tile([2, P], F32, name="negblk")
            nc.vector.memset(negblk, 0.0)
            nc.vector.memset(negblk[0:1, 0:P // 2], -1.0)
            nc.vector.memset(negblk[1:2, P // 2:P], -1.0)
            # [P,2] blockmask: col 0 = 1 on partitions 0:64, col 1 on 64:128
            blk = const_pool.tile([P, 2], F32, name="blk")
            nc.vector.memset(blk, 0.0)
            nc.vector.memset(blk[0:P // 2, 0:1], 1.0)
            nc.vector.memset(blk[P // 2:P, 1:2], 1.0)
            # per-partition masks for splitting p into halves (tensor_scalar
            # requires a float32 scalar operand)
            mask_top = const_pool.tile([P, 1], F32, name="mask_top")
            nc.vector.memset(mask_top, 0.0)
            nc.vector.memset(mask_top[0:P // 2, :], 1.0)
            mask_bot = const_pool.tile([P, 1], F32, name="mask_bot")
            nc.vector.memset(mask_bot, 0.0)
            nc.vector.memset(mask_bot[P // 2:P, :], 1.0)

            # ---- q = ht @ W (identical to build_nc) ----
            w_sb = const_pool.tile([P, hc, H], F32, name="w_sb")
            nc.sync.dma_start(
                out=w_sb, in_=w_d[:].rearrange("(c p) k -> p c k", p=P)
            )
            htT = const_pool.tile([P, hc, b_loc], F32, name="htT")
            for c in range(hc):
                nc.gpsimd.dma_start(
                    out=htT[:, c, :],
                    in_=ht_d[:, 0, c * P : (c + 1) * P].rearrange("b p -> p b"),
                )
            dmy_ps = psum_pool.tile(
                [b_loc, b_loc], F32, name="dmy_ps", tag="dmy", bufs=1
            )
            nc.tensor.matmul(
                dmy_ps, lhsT=htT[:, 0, :], rhs=htT[:, 0, :], start=True, stop=True
            )
            q_ps = psum_pool.tile([b_loc, H], F32, name="q_ps", tag="qps")
            for c in range(hc):
                nc.tensor.matmul(
                    q_ps, lhsT=htT[:, c, :], rhs=w_sb[:, c, :],
                    start=(c == 0), stop=(c == hc - 1),
                )
            q_sb = const_pool.tile([b_loc, H], F32, name="q_sb")
            nc.vector.tensor_copy(q_sb, q_ps)
            q_dram = dram_pool.tile([b_loc, H], F32, name="q_dram")
            nc.sync.dma_start(out=q_dram, in_=q_sb)
            # q_pair[p, c, :] = q[2c + (p >= 64)]
            q_pair = const_pool.tile([P, n_pairs, H], F32, name="q_pair")
            nc.sync.dma_start(
                out=q_pair[0:P // 2, :, :],
                in_=bass.AP(
                    tensor=q_dram.tensor, offset=q_dram.offset,
                    ap=[[0, P // 2], [2 * H, n_pairs], [1, H]],
                ),
            )
            nc.sync.dma_start(
                out=q_pair[P // 2:P, :, :],
                in_=bass.AP(
                    tensor=q_dram.tensor, offset=q_dram.offset + H,
                    ap=[[0, P // 2], [2 * H, n_pairs], [1, H]],
                ),
            )
            q_pair_bf = const_pool.tile([P, n_pairs, H], BF16, name="q_pair_bf")
            nc.vector.tensor_copy(q_pair_bf, q_pair)

            # ---- pair loop ----
            for c in [cc for _ in range(reps) for cc in range(n_pairs)]:
                s_all = stats_pool.tile([P, tpb2], F32, name="s_all", tag="s_all")
                hch = h_pool.tile([P, tpb2, H], BF16, name="hch", tag="hch")
                if split_dma:
                    # two 8 MiB DMAs into the same tile so scores on the
                    # first half can start at the halfway point
                    half = tpb2 // 2
                    for j in range(2):
                        nc.gpsimd.dma_start(
                            out=hch[:, j * half:(j + 1) * half, :],
                            in_=bass.AP(
                                tensor=h_flat.tensor,
                                offset=h_flat.offset
                                + (c * 2 * t) * H + j * half * H,
                                ap=[[tpb2 * H, P], [H, half], [1, H]],
                            ),
                        )
                else:
                    nc.gpsimd.dma_start(
                        out=hch,
                        in_=h_flat[c * 2 * t:(c + 1) * 2 * t, :].rearrange(
                            "(p k) j -> p k j", k=tpb2),
                    )
                for k in range(tpb2):
                    vscr = scr_pool.tile([P, H], BF16, name="vscr", tag="vscr")
                    nc.vector.scalar_tensor_tensor(
                        out=vscr, in0=hch[:, k, :],
                        scalar=1.0, in1=q_pair_bf[:, c, :],
                        op0=mybir.AluOpType.mult,
                        op1=mybir.AluOpType.mult,
                        accum_out=s_all[:, k:k + 1],
                    )

                # ---- softmax statistics (per batch-half) ----
                m_col = stats_pool.tile([P, 1], F32, name="m_col", tag="m_col")
                nc.vector.reduce_max(m_col, s_all, axis=mybir.AxisListType.X)
                mT_ps = psum_pool.tile([1, P], F32, name="mT_ps", tag="mT")
                nc.tensor.transpose(mT_ps, m_col, ident)
                m01 = stats_pool.tile([1, 2], F32, name="m01", tag="m01")
                nc.vector.reduce_max(
                    m01[:, 0:1], mT_ps[:, 0:P // 2], axis=mybir.AxisListType.X
                )
                nc.vector.reduce_max(
                    m01[:, 1:2], mT_ps[:, P // 2:P], axis=mybir.AxisListType.X
                )
                mp_ps = psum_pool.tile([2, 1], F32, name="mp_ps", tag="mp")
                nc.tensor.transpose(mp_ps, m01, ident[0:1, 0:1])
                mp_sb = stats_pool.tile([2, 1], F32, name="mp_sb", tag="mp_sb")
                nc.vector.tensor_copy(mp_sb, mp_ps)
                negm_ps = psum_pool.tile([P, 1], F32, name="negm_ps", tag="negm")
                nc.tensor.matmul(
                    negm_ps, lhsT=negblk, rhs=mp_sb, start=True, stop=True
                )
                negm_sb = stats_pool.tile([P, 1], F32, name="negm_sb", tag="negm_sb")
                nc.vector.tensor_copy(negm_sb, negm_ps)
                p_all = stats_pool.tile([P, tpb2], BF16, name="p_all", tag="p_all")
                l_col = stats_pool.tile([P, 1], F32, name="l_col", tag="l_col")
                nc.scalar.activation(
                    out=p_all, in_=s_all, func=mybir.ActivationFunctionType.Exp,
                    bias=negm_sb, scale=1.0, accum_out=l_col,
                )
                l_ps = psum_pool.tile([2, 1], F32, name="l_ps", tag="l")
                nc.tensor.matmul(
                    l_ps, lhsT=blk, rhs=l_col, start=True, stop=True
                )
                inv_l = stats_pool.tile([2, 1], F32, name="inv_l", tag="inv_l")
                nc.vector.reciprocal(inv_l, l_ps)

                # ---- weighted sum for both halves in one matmul chain ----
                p_pair = stats_pool.tile(
                    [P, 2, tpb2], BF16, name="p_pair", tag="p_pair"
                )
                nc.vector.tensor_scalar(
                    p_pair[:, 0, :], p_all, mask_top, None,
                    op0=mybir.AluOpType.mult,
                )
                nc.vector.tensor_scalar(
                    p_pair[:, 1, :], p_all, mask_bot, None,
                    op0=mybir.AluOpType.mult,
                )
                ct_ps = psum_pool.tile([2, H], F32, name="ct_ps", tag="ct", bufs=2)
                for k in range(tpb2):
                    nc.tensor.matmul(
                        ct_ps, lhsT=p_pair[:, :, k],
                        rhs=hch[:, k, :],
                        start=(k == 0), stop=(k == tpb2 - 1),
                    )
                ct_sb = out_pool.tile([2, H], F32, name="ct_sb", tag="ct_sb")
                nc.vector.tensor_scalar(
                    ct_sb, ct_ps, inv_l, None, op0=mybir.AluOpType.mult
                )
                nc.sync.dma_start(out=out_d[2 * c:2 * c + 2, 0, :], in_=ct_sb)

    if not nc.is_finalized():
        nc.finalize()
    return nc


def build_nc(b_loc=B_LOC, t=T, chunk_k=64, h_bufs=5, reps=1,
             dual_ring=True, out_ring="sync", tri_ring=False,
             score_stride=1, bf16_chunks=True, out_batch=True,
             mix_chunks=False, mix_bufs=(3, 3)):
    """Build the per-core Bass graph.

    chunk_k: T-tiles (of 128 timesteps) per DMA chunk.
    h_bufs:  chunk-pool slots (h_bufs * chunk_k * 128 timesteps resident).
    reps:    unrolled repetitions of the whole batch loop (benchmarking
             aid — wall(reps=3) - wall(reps=1) = 2x the kernel time,
             cancelling dispatch overhead).
    dual_ring: alternate chunk DMAs between the SP and ACT HWDGE rings.
    out_ring: 'sync' or 'scalar' — ring for the per-batch output DMA.
    out_batch: stage all b_loc ct rows in one partition-0 SBUF tile and
             write them with a single 8 KB DMA per rep instead of
             b_loc separate 1 KB DMAs.
    """
    tpb = t // P              # T-tiles per batch
    n_chunks = tpb // chunk_k
    hc = H // P               # contraction chunks for the q matmul

    from concourse.bacc import Bacc

    nc = Bacc()
    h_d = nc.declare_dram_parameter("h_0_t", [b_loc, t, H], F32, isOutput=False)
    ht_d = nc.declare_dram_parameter("ht", [b_loc, 1, H], F32, isOutput=False)
    w_d = nc.declare_dram_parameter("weight", [H, H], F32, isOutput=False)
    out_d = nc.declare_dram_parameter("out", [b_loc, 1, H], F32, isOutput=True)

    with TileContext(nc) as tc:
        with (
            tc.tile_pool(name="const", bufs=1) as const_pool,
            tc.tile_pool(name="hbuf", bufs=h_bufs) as h_pool,
            tc.tile_pool(name="stats", bufs=2) as stats_pool,
            tc.tile_pool(name="scr", bufs=2) as scr_pool,
            tc.tile_pool(name="ctout", bufs=2) as out_pool,
            tc.tile_pool(name="ps", bufs=1, space="PSUM") as psum_pool,
            tc.tile_pool(name="qdram", bufs=1, space="DRAM") as dram_pool,
        ):
            # ---- constants ----
            ones_col = const_pool.tile([P, 1], F32, name="ones_col")
            nc.vector.memset(ones_col, 1.0)
            neg_ones_row = const_pool.tile([1, P], F32, name="neg_ones_row")
            nc.vector.memset(neg_ones_row, -1.0)
            ident_i = const_pool.tile([P, P], I32, name="ident_i")
            nc.gpsimd.iota(ident_i, pattern=[[-1, P]], base=0, channel_multiplier=1)
            ident = const_pool.tile([P, P], F32, name="ident")
            nc.vector.tensor_scalar(
                ident, ident_i, 0, None, op0=mybir.AluOpType.is_equal
            )

            # ---- q = ht @ W for all local batches (one-time setup) ----
            w_sb = const_pool.tile([P, hc, H], F32, name="w_sb")
            nc.sync.dma_start(
                out=w_sb, in_=w_d[:].rearrange("(c p) k -> p c k", p=P)
            )
            htT = const_pool.tile([P, hc, b_loc], F32, name="htT")
            for c in range(hc):
                nc.gpsimd.dma_start(
                    out=htT[:, c, :],
                    in_=ht_d[:, 0, c * P : (c + 1) * P].rearrange("b p -> p b"),
                )
            # dummy self-matmul absorbs the htT DMA wait so the q matmul
            # carries a single sync wait (PE LDWEIGHTS allows only one)
            dmy_ps = psum_pool.tile(
                [b_loc, b_loc], F32, name="dmy_ps", tag="dmy", bufs=2
            )
            nc.tensor.matmul(
                dmy_ps, lhsT=htT[:, 0, :], rhs=htT[:, 0, :], start=True, stop=True
            )
            q_ps = psum_pool.tile([b_loc, H], F32, name="q_ps", tag="qps")
            for c in range(hc):
                nc.tensor.matmul(
                    q_ps, lhsT=htT[:, c, :], rhs=w_sb[:, c, :],
                    start=(c == 0), stop=(c == hc - 1),
                )
            q_sb = const_pool.tile([b_loc, H], F32, name="q_sb")
            nc.vector.tensor_copy(q_sb, q_ps)
            # replicate each batch's q across all 128 partitions (DRAM bounce)
            q_dram = dram_pool.tile([b_loc, H], F32, name="q_dram")
            nc.sync.dma_start(out=q_dram, in_=q_sb)
            q_rep = const_pool.tile([P, b_loc, H], F32, name="q_rep")
            q_bcast_src = bass.AP(
                tensor=q_dram.tensor, offset=q_dram.offset,
                ap=[[0, P], [H, b_loc], [1, H]],
            )
            nc.sync.dma_start(out=q_rep, in_=q_bcast_src)
            if bf16_chunks:
                BF16 = mybir.dt.float16
                q_rep_bf = const_pool.tile(
                    [P, b_loc, H], BF16, name="q_rep_bf"
                )
                nc.vector.tensor_copy(q_rep_bf, q_rep)

            # ---- batch loop ----
            ct_all = None
            for b in [bb for _ in range(reps) for bb in range(b_loc)]:
                if out_batch and b == 0:
                    ct_all = out_pool.tile(
                        [1, b_loc, H], F32, name="ct_all", tag="ct_all"
                    )
                s_all = stats_pool.tile([P, tpb], F32, name="s_all", tag="s_all")
                if mix_chunks:
                    # Half the batch as fp16 via SWDGE (gpsimd, cast in the
                    # SDMA datapath), half as raw f32r via HWDGE
                    # (sync/scalar alternating): two DMA queues pull
                    # concurrently, which wins slightly more HBM arbitration
                    # share when the stack is contended.
                    th = tpb // 2
                    BF16 = mybir.dt.float16
                    hch_a = h_pool.tile(
                        [P, th, H], BF16, name="hch_a", tag="hcha",
                        bufs=mix_bufs[0],
                    )
                    nc.gpsimd.dma_start(
                        out=hch_a,
                        in_=h_d[b, 0:t // 2, :].rearrange(
                            "(p k) j -> p k j", k=th),
                    )
                    hch_b = h_pool.tile(
                        [P, th, H], F32R, name="hch_b", tag="hchb",
                        bufs=mix_bufs[1],
                    )
                    eng = nc.scalar if b % 2 else nc.sync
                    eng.dma_start(
                        out=hch_b,
                        in_=h_d[b, t // 2:, :].rearrange(
                            "(p k) j -> p k j", k=th).bitcast(F32R),
                    )
                    for k in range(th):
                        vscr = scr_pool.tile(
                            [P, H], BF16, name="vscr", tag="vscr"
                        )
                        nc.vector.scalar_tensor_tensor(
                            out=vscr, in0=hch_a[:, k, :],
                            scalar=1.0, in1=q_rep_bf[:, b, :],
                            op0=mybir.AluOpType.mult,
                            op1=mybir.AluOpType.mult,
                            accum_out=s_all[:, k:k + 1],
                        )
                    for k in range(th):
                        vscrf = scr_pool.tile(
                            [P, H], F32, name="vscrf", tag="vscrf"
                        )
                        nc.vector.scalar_tensor_tensor(
                            out=vscrf, in0=hch_b[:, k, :].bitcast(F32),
                            scalar=1.0, in1=q_rep[:, b, :],
                            op0=mybir.AluOpType.mult,
                            op1=mybir.AluOpType.mult,
                            accum_out=s_all[:, th + k:th + k + 1],
                        )
                    chunks = [(hch_a, hch_b)]
                else:
                    chunks = []
                for c in range(n_chunks) if not mix_chunks else []:
                    # declared fp32r so TensorE can consume it at full rate;
                    # the DVE scores path reads the same bits as fp32.
                    # Blocked T layout: partition p holds chunk_k CONSECUTIVE
                    # timesteps (contiguous chunk_k*1KB DRAM per partition ->
                    # large DMA descriptors). softmax + weighted sum are
                    # permutation-invariant over T, so the order change is
                    # harmless.
                    src = h_d[b, c * chunk_k * P : (c + 1) * chunk_k * P, :]
                    if bf16_chunks:
                        # SWDGE casts f32->fp16 inside the SDMA datapath
                        # (free); HBM read side is unchanged, SBUF write
                        # side halves, DVE scores run at 2x on 16-bit and
                        # PE consumes bf16 at full rate.
                        hch = h_pool.tile(
                            [P, chunk_k, H], mybir.dt.float16,
                            name="hch", tag="hch",
                        )
                        nc.gpsimd.dma_start(
                            out=hch,
                            in_=src.rearrange("(p k) j -> p k j", k=chunk_k),
                        )
                    else:
                        hch = h_pool.tile(
                            [P, chunk_k, H], F32R, name="hch", tag="hch"
                        )
                        if tri_ring:
                            dma_eng = [nc.sync, nc.scalar, nc.gpsimd][c % 3]
                        else:
                            dma_eng = (
                                nc.scalar if (dual_ring and c % 2) else nc.sync
                            )
                        dma_eng.dma_start(
                            out=hch,
                            in_=src.rearrange(
                                "(p k) j -> p k j", k=chunk_k
                            ).bitcast(F32R),
                        )
                    chunks.append(hch)
                    for k in range(0, chunk_k, score_stride):
                        gk = c * chunk_k + k
                        if bf16_chunks:
                            vscr = scr_pool.tile(
                                [P, H], mybir.dt.float16, name="vscr",
                                tag="vscr",
                            )
                            nc.vector.scalar_tensor_tensor(
                                out=vscr, in0=hch[:, k, :],
                                scalar=1.0, in1=q_rep_bf[:, b, :],
                                op0=mybir.AluOpType.mult,
                                op1=mybir.AluOpType.mult,
                                accum_out=s_all[:, gk : gk + 1],
                            )
                        else:
                            vscr = scr_pool.tile(
                                [P, H], F32, name="vscr", tag="vscr"
                            )
                            nc.vector.scalar_tensor_tensor(
                                out=vscr, in0=hch[:, k, :].bitcast(F32),
                                scalar=1.0, in1=q_rep[:, b, :],
                                op0=mybir.AluOpType.mult,
                                op1=mybir.AluOpType.mult,
                                accum_out=s_all[:, gk : gk + 1],
                            )

                # ---- softmax statistics ----
                m_col = stats_pool.tile([P, 1], F32, name="m_col", tag="m_col")
                nc.vector.reduce_max(m_col, s_all, axis=mybir.AxisListType.X)
                mT_ps = psum_pool.tile([1, P], F32, name="mT_ps", tag="mT")
                nc.tensor.transpose(mT_ps, m_col, ident)
                m_sb = stats_pool.tile([1, 1], F32, name="m_sb", tag="m_sb")
                nc.vector.reduce_max(m_sb, mT_ps, axis=mybir.AxisListType.X)
                # broadcast -max to all partitions via a C=1 matmul
                negm_ps = psum_pool.tile([P, 1], F32, name="negm_ps", tag="negm")
                nc.tensor.matmul(
                    negm_ps, lhsT=neg_ones_row, rhs=m_sb, start=True, stop=True
                )
                negm_sb = stats_pool.tile([P, 1], F32, name="negm_sb", tag="negm_sb")
                nc.vector.tensor_copy(negm_sb, negm_ps)
                p_dt = mybir.dt.float16 if bf16_chunks else F32R
                p_all = stats_pool.tile([P, tpb], p_dt, name="p_all", tag="p_all")
                l_col = stats_pool.tile([P, 1], F32, name="l_col", tag="l_col")
                nc.scalar.activation(
                    out=p_all, in_=s_all, func=mybir.ActivationFunctionType.Exp,
                    bias=negm_sb, scale=1.0, accum_out=l_col,
                )
                l_ps = psum_pool.tile([1, 1], F32, name="l_ps", tag="l")
                nc.tensor.matmul(
                    l_ps, lhsT=l_col, rhs=ones_col, start=True, stop=True
                )
                inv_l = stats_pool.tile([1, 1], F32, name="inv_l", tag="inv_l")
                nc.vector.reciprocal(inv_l, l_ps)

                # ---- weighted sum over T on TensorE ----
                ct_ps = psum_pool.tile([1, H], F32, name="ct_ps", tag="ct", bufs=2)
                if mix_chunks:
                    th = tpb // 2
                    hch_a, hch_b = chunks[0]
                    # f32r copy of the f32-half weights so lhsT/rhs dtypes
                    # match within each matmul
                    p_r = stats_pool.tile([P, th], F32R, name="p_r", tag="p_r")
                    nc.vector.tensor_copy(p_r, p_all[:, th:tpb])
                    for k in range(th):
                        nc.tensor.matmul(
                            ct_ps, lhsT=p_all[:, k:k + 1],
                            rhs=hch_a[:, k, :],
                            start=(k == 0), stop=False,
                        )
                    for k in range(th):
                        nc.tensor.matmul(
                            ct_ps, lhsT=p_r[:, k:k + 1],
                            rhs=hch_b[:, k, :],
                            start=False, stop=(k == th - 1),
                        )
                else:
                    for c in range(n_chunks):
                        for k in range(chunk_k):
                            gk = c * chunk_k + k
                            nc.tensor.matmul(
                                ct_ps, lhsT=p_all[:, gk : gk + 1],
                                rhs=chunks[c][:, k, :],
                                start=(gk == 0), stop=(gk == tpb - 1),
                            )
                out_eng = nc.scalar if out_ring == "scalar" else nc.sync
                if out_batch:
                    nc.vector.tensor_scalar_mul(
                        ct_all[:, b, :], ct_ps, inv_l[0:1, 0:1]
                    )
                    if b == b_loc - 1:
                        out_eng.dma_start(
                            out=out_d[:].rearrange("b o h -> o b h"),
                            in_=ct_all,
                        )
                else:
                    ct_sb = out_pool.tile([1, H], F32, name="ct_sb", tag="ct_sb")
                    nc.vector.tensor_scalar_mul(ct_sb, ct_ps, inv_l[0:1, 0:1])
                    out_eng.dma_start(out=out_d[b, :, :], in_=ct_sb)

    # Bacc.finalize() runs the lowering passes raw Bass lacks: matmul-wait
    # relocation, event-semaphore wait splitting (HW allows 1 wait/inst),
    # GPSIMD library loads, ACT table loads, and extended-ISA codegen.
    if not nc.is_finalized():
        nc.finalize()
    return nc


_nc_cache = None


def _get_nc():
    global _nc_cache
    if _nc_cache is None:
        _nc_cache = build_nc()
    return _nc_cache


def _run(inputs, trace=False, nc=None, **kw):
    if nc is None:
        nc = _get_nc()
    ht = np.ascontiguousarray(np.asarray(inputs["ht"], dtype=np.float32))
    h0 = np.asarray(inputs["h_0_t"], dtype=np.float32)
    w = np.ascontiguousarray(np.asarray(inputs["weight"], dtype=np.float32))
    in_maps = []
    for i in range(N_CORES):
        sl = slice(i * B_LOC, (i + 1) * B_LOC)
        in_maps.append(
            {
                "h_0_t": np.ascontiguousarray(h0[sl]),
                "ht": np.ascontiguousarray(ht[sl]),
                "weight": w,
            }
        )
    res = run_bass_kernel_spmd(
        nc, in_maps, core_ids=list(range(N_CORES)), trace=trace, **kw
    )
    out = np.concatenate([r["out"] for r in res.results], axis=0)
    return out, res


def kernel(**inputs):
    out, _ = _run(inputs)
    return out


# ---------------------------------------------------------------------------
# Timing helper (used by test.py only; not part of the grading contract).
# Rebuilds the shard_map executable once so repeat calls reuse one compiled
# NEFF with device-resident inputs, then reports min wall-clock.
# ---------------------------------------------------------------------------


_nc_rep_cache = {}


def _get_exec(inputs, reps=1):
    """Build (once) and return a zero-arg callable running the reps-unrolled
    kernel on all 8 cores with device-resident inputs."""
    import jax
    from jax.experimental.shard_map import shard_map
    from jax.sharding import Mesh, NamedSharding, PartitionSpec

    from concourse import bass2jax

    if reps == 1:
        nc = _get_nc()
    else:
        if reps not in _nc_rep_cache:
            _nc_rep_cache[reps] = build_nc(reps=reps)
        nc = _nc_rep_cache[reps]
    bass2jax.install_neuronx_cc_hook()

    partition_name = (
        nc.partition_id_tensor.name if nc.partition_id_tensor else None
    )
    in_names, out_names, out_avals, zero_outs = [], [], [], []
    for alloc in nc.m.functions[0].allocations:
        if not isinstance(alloc, mybir.MemoryLocationSet):
            continue
        name = alloc.memorylocations[0].name
        if alloc.kind == "ExternalInput":
            if name != partition_name:
                in_names.append(name)
        elif alloc.kind == "ExternalOutput":
            out_names.append(name)
            shape = tuple(alloc.tensor_shape)
            dtype = mybir.dt.np(alloc.dtype)
            out_avals.append(jax.core.ShapedArray(shape, dtype))
            zero_outs.append(np.zeros(shape, dtype))
    n_params = len(in_names)
    n_outs = len(out_avals)
    all_names = list(in_names) + out_names
    if partition_name is not None:
        all_names.append(partition_name)

    def _body(*args):
        operands = list(args)
        if partition_name is not None:
            operands.append(bass2jax.partition_id_tensor())
        outs = bass2jax._bass_exec_p.bind(
            *operands,
            out_avals=tuple(out_avals),
            in_names=tuple(all_names),
            out_names=tuple(out_names),
            lowering_input_output_aliases=(),
            sim_require_finite=True,
            sim_require_nnan=True,
            nc=nc,
        )
        return tuple(outs)

    devices = jax.devices()[:N_CORES]
    mesh = Mesh(np.asarray(devices), ("core",))
    in_specs = (PartitionSpec("core"),) * (n_params + n_outs)
    out_specs = (PartitionSpec("core"),) * n_outs
    sharded = jax.jit(
        shard_map(
            _body, mesh=mesh, in_specs=in_specs, out_specs=out_specs,
            check_rep=False,
        ),
        keep_unused=True,
    )

    ht = np.ascontiguousarray(np.asarray(inputs["ht"], dtype=np.float32))
    h0 = np.ascontiguousarray(np.asarray(inputs["h_0_t"], dtype=np.float32))
    w = np.asarray(inputs["weight"], dtype=np.float32)
    per_core = {
        "h_0_t": h0,
        "ht": ht,
        "weight": np.concatenate([w[None]] * N_CORES, axis=0).reshape(
            N_CORES * w.shape[0], w.shape[1]
        ),
    }
    sh = NamedSharding(mesh, PartitionSpec("core"))
    xs = [jax.device_put(per_core[name], sh) for name in in_names]
    zs = [
        jax.device_put(
            np.zeros((N_CORES * z.shape[0], *z.shape[1:]), z.dtype), sh
        )
        for z in zero_outs
    ]

    def call():
        jax.block_until_ready(sharded(*xs, *zs))

    call()  # warm up (includes compile)
    return call


def time_kernel_pair(inputs, iters=60, reps_hi=3, reps_lo=1):
    """Interleaved slope timing: min(wall_hi) - min(wall_lo) over paired
    adjacent samples cancels axon dispatch overhead and its drift.
    Returns one kernel execution time in ns."""
    import time

    lo = _get_exec(inputs, reps=reps_lo)
    hi = _get_exec(inputs, reps=reps_hi)
    t_lo, t_hi = [], []
    for _ in range(iters):
        t0 = time.perf_counter()
        lo()
        t1 = time.perf_counter()
        hi()
        t2 = time.perf_counter()
        t_lo.append(t1 - t0)
        t_hi.append(t2 - t1)
    ns = (min(t_hi) - min(t_lo)) / (reps_hi - reps_lo) * 1e9
    return ns, min(t_lo) * 1e9, min(t_hi) * 1e9



# revision 35
# speedup vs baseline: 1.0772x; 1.0037x over previous
"""Trainium2 Bass kernel for a single-query attention layer.

Reference computation (per batch b):
    q      = ht[b] @ W                      # (1, H)
    scores = q . h_0_t[b, t, :] over H      # (T,)
    alpha  = softmax(scores)                # (T,)
    ct[b]  = sum_t alpha[t] * h_0_t[b, t]   # (1, H)

Sharding: data-parallel over batch across 8 NeuronCores (8 batches per
core); the (H, H) weight is replicated.  No collectives.

Per-core dataflow (memory-bound; the 64 MiB h_0_t shard is read from
HBM exactly once):
  - stream h_0_t[b] as one 8 MiB whole-batch DMA into SBUF, natural
    [T-on-partitions, H-on-free] layout (the only DMA-efficient one for
    this DRAM layout); big chunks amortize SWDGE per-DMA Q7 descriptor
    emission (in the non-default f32 path, chunk DMAs instead alternate
    between the two HWDGE rings, measured -17% vs all-chunks-on-SP)
  - chunks land as fp16: SWDGE casts f32->fp16 inside the SDMA datapath
    (HBM reads unchanged, SBUF writes halve, DVE scores run at 2x,
    PE consumes fp16 at full rate; rel err 3.8e-3 vs 2e-2 tolerance)
  - scores: one fused multiply+reduce (tensor_tensor_reduce) per
    128-timestep tile on VectorE, against a partition-replicated q
  - softmax: free-axis max (DVE) -> partition max via PE transpose +
    free-axis max -> exp with fused bias and fused row-sum (ScalarE) ->
    partition sum via a ones-matmul (PE)
  - weighted sum: 64 TensorE matmuls per batch (contraction over T =
    partition axis, which the natural layout supports directly), fp16
    at full PE rate, accumulated in PSUM
  - scale by 1/denominator during the PSUM->SBUF copy into a staging
    row; all 8 ct rows leave in ONE 8 KB DMA per pass (out_batch)
    instead of eight 1 KB DMAs, and the chunk pool holds 5 batches
    (h_bufs=5) so several batch DMAs are queued ahead of compute.
    Both measured neutral (+/-0.3 us/pass) in order-rotated paired A/B
    under HBM contention; adopted because they strictly reduce DMA-ring
    entries / deepen prefetch at zero cost.

Rejected by order-rotated paired A/B (same-round interleaved hi-reps
bursts, median paired delta over 45-60 rounds): half-fp16/half-f32r
dual-queue chunks (+5.6 us/pass in-kernel), 16 MiB two-batch chunks
(+2.2 us/pass as pure DMA), dual/tri-ring f32 chunk placement (neutral
to +1.6).  NOTE: un-rotated rounds have a ~2-3 us position-in-round
bias -- rotate variant order or the A/B lies.  The DMA stream sits at
the pairwise-stack HBM floor (~358 GB/s/core, 712 GB/s/stack measured
= 99.5% of the 716 GB/s spec); a pure-DMA-only graph runs ~1 us/pass
(mild load) to ~5 us/pass (heavy co-tenant load) faster than the full
kernel, ~80% of it owned by the scores DVE stream (measured by a
dma-plus-scores-only decomposition graph).  That residual is closed to
further optimization by the TRN2 ISA: neither the scores op's dead
[P,H] main-out write (tensor2d_valid) nor the replicated-q operand
reads (tensor4d_valid: DVE cannot write 16-bit PSUM) can be moved off
SBUF, so the scores stage's ~12 MiB/batch of SBUF traffic vs the DMA's
4 MiB/batch write stream is irreducible.
"""

import sys

import numpy as np

_BASS_ROOT = "/opt/trn_rl_repo"
if _BASS_ROOT not in sys.path:
    sys.path.insert(0, _BASS_ROOT)

import concourse.bass as bass  # noqa: E402
from concourse import mybir  # noqa: E402
from concourse.bass_utils import run_bass_kernel_spmd  # noqa: E402
from concourse.tile import TileContext  # noqa: E402

B, T, H = 64, 8192, 256
N_CORES = 8
B_LOC = B // N_CORES  # batches per core
P = 128               # SBUF partitions
F32 = mybir.dt.float32
F32R = mybir.dt.float32r
I32 = mybir.dt.int32


def build_nc_pair(b_loc=B_LOC, t=T, reps=1, h_bufs=2, split_dma=False):
    """EXPERIMENTAL — not used by kernel().  The 16 MiB pair-chunk scheme
    measured +2.2 us/pass slower than 8 MiB chunks as pure DMA under HBM
    contention, and this revision still fails neuronxcc compilation;
    retained for reference only.

    Pair-chunk variant: one 16 MiB SWDGE fp16-cast DMA covers TWO
    batches (batch 2c on partitions 0-63, batch 2c+1 on 64-127), halving
    the number of DMA-queue entries vs build_nc.  Per-batch engine costs
    are kept equal to build_nc:

      - scores: one DVE inst per timestep-column across all 128 partitions,
        against q_pair (q[2c] replicated on the top half, q[2c+1] on the
        bottom half)
      - softmax max/sum are computed per batch-half via block-mask matmuls
        (a [2,P] neg-blockmask broadcasts the two maxima back to their
        partition halves; a [P,2] blockmask row-sums l per half)
      - weighted sum: one matmul per column with M=2 (lhsT = [P,2] with
        the top/bottom halves of p masked), accumulating ct for both
        batches into a [2,H] PSUM tile
    """
    tpb2 = 2 * (t // P)       # score columns per pair
    n_pairs = b_loc // 2
    hc = H // P

    from concourse.bacc import Bacc

    nc = Bacc()
    h_d = nc.declare_dram_parameter("h_0_t", [b_loc, t, H], F32, isOutput=False)
    ht_d = nc.declare_dram_parameter("ht", [b_loc, 1, H], F32, isOutput=False)
    w_d = nc.declare_dram_parameter("weight", [H, H], F32, isOutput=False)
    out_d = nc.declare_dram_parameter("out", [b_loc, 1, H], F32, isOutput=True)
    h_flat = h_d[:].rearrange("b t j -> (b t) j")
    BF16 = mybir.dt.float16

    with TileContext(nc) as tc:
        with (
            tc.tile_pool(name="const", bufs=1) as const_pool,
            tc.tile_pool(name="hbuf", bufs=h_bufs) as h_pool,
            tc.tile_pool(name="stats", bufs=2) as stats_pool,
            tc.tile_pool(name="scr", bufs=2) as scr_pool,
            tc.tile_pool(name="ctout", bufs=2) as out_pool,
            tc.tile_pool(name="ps", bufs=1, space="PSUM") as psum_pool,
            tc.tile_pool(name="qdram", bufs=1, space="DRAM") as dram_pool,
        ):
            # ---- constants ----
            ones_col = const_pool.tile([P, 1], F32, name="ones_col")
            nc.vector.memset(ones_col, 1.0)
            neg_ones_row = const_pool.tile([1, P], F32, name="neg_ones_row")
            nc.vector.memset(neg_ones_row, -1.0)
            ident_i = const_pool.tile([P, P], I32, name="ident_i")
            nc.gpsimd.iota(ident_i, pattern=[[-1, P]], base=0, channel_multiplier=1)
            ident = const_pool.tile([P, P], F32, name="ident")
            nc.vector.tensor_scalar(
                ident, ident_i, 0, None, op0=mybir.AluOpType.is_equal
            )
            # [2,P] neg-blockmask: row 0 = -1 on cols 0:64, row 1 = -1 on 64:128
            negblk = const_pool.tile([2, P], F32, name="negblk")
            nc.vector.memset(negblk, 0.0)
            nc.vector.memset(negblk[0:1, 0:P // 2], -1.0)
            nc.vector.memset(negblk[1:2, P // 2:P], -1.0)
            # [P,2] blockmask: col 0 = 1 on partitions 0:64, col 1 on 64:128
            blk = const_pool.tile([P, 2], F32, name="blk")
            nc.vector.memset(blk, 0.0)
            nc.vector.memset(blk[0:P // 2, 0:1], 1.0)
            nc.vector.memset(blk[P // 2:P, 1:2], 1.0)
            # per-partition masks for splitting p into halves (tensor_scalar
            # requires a float32 scalar operand)
            mask_top = const_pool.tile([P, 1], F32, name="mask_top")
            nc.vector.memset(mask_top, 0.0)
            nc.vector.memset(mask_top[0:P // 2, :], 1.0)
            mask_bot = const_pool.tile([P, 1], F32, name="mask_bot")
            nc.vector.memset(mask_bot, 0.0)
            nc.vector.memset(mask_bot[P // 2:P, :], 1.0)

            # ---- q = ht @ W (identical to build_nc) ----
            w_sb = const_pool.tile([P, hc, H], F32, name="w_sb")
            nc.sync.dma_start(
                out=w_sb, in_=w_d[:].rearrange("(c p) k -> p c k", p=P)
            )
            htT = const_pool.tile([P, hc, b_loc], F32, name="htT")
            for c in range(hc):
                nc.gpsimd.dma_start(
                    out=htT[:, c, :],
                    in_=ht_d[:, 0, c * P : (c + 1) * P].rearrange("b p -> p b"),
                )
            dmy_ps = psum_pool.tile(
                [b_loc, b_loc], F32, name="dmy_ps", tag="dmy", bufs=1
            )
            nc.tensor.matmul(
                dmy_ps, lhsT=htT[:, 0, :], rhs=htT[:, 0, :], start=True, stop=True
            )
            q_ps = psum_pool.tile([b_loc, H], F32, name="q_ps", tag="qps")
            for c in range(hc):
                nc.tensor.matmul(
                    q_ps, lhsT=htT[:, c, :], rhs=w_sb[:, c, :],
                    start=(c == 0), stop=(c == hc - 1),
                )
            q_sb = const_pool.tile([b_loc, H], F32, name="q_sb")
            nc.vector.tensor_copy(q_sb, q_ps)
            q_dram = dram_pool.tile([b_loc, H], F32, name="q_dram")
            nc.sync.dma_start(out=q_dram, in_=q_sb)
            # q_pair[p, c, :] = q[2c + (p >= 64)]
            q_pair = const_pool.tile([P, n_pairs, H], F32, name="q_pair")
            nc.sync.dma_start(
                out=q_pair[0:P // 2, :, :],
                in_=bass.AP(
                    tensor=q_dram.tensor, offset=q_dram.offset,
                    ap=[[0, P // 2], [2 * H, n_pairs], [1, H]],
                ),
            )
            nc.sync.dma_start(
                out=q_pair[P // 2:P, :, :],
                in_=bass.AP(
                    tensor=q_dram.tensor, offset=q_dram.offset + H,
                    ap=[[0, P // 2], [2 * H, n_pairs], [1, H]],
                ),
            )
            q_pair_bf = const_pool.tile([P, n_pairs, H], BF16, name="q_pair_bf")
            nc.vector.tensor_copy(q_pair_bf, q_pair)

            # ---- pair loop ----
            for c in [cc for _ in range(reps) for cc in range(n_pairs)]:
                s_all = stats_pool.tile([P, tpb2], F32, name="s_all", tag="s_all")
                hch = h_pool.tile([P, tpb2, H], BF16, name="hch", tag="hch")
                if split_dma:
                    # two 8 MiB DMAs into the same tile so scores on the
                    # first half can start at the halfway point
                    half = tpb2 // 2
                    for j in range(2):
                        nc.gpsimd.dma_start(
                            out=hch[:, j * half:(j + 1) * half, :],
                            in_=bass.AP(
                                tensor=h_flat.tensor,
                                offset=h_flat.offset
                                + (c * 2 * t) * H + j * half * H,
                                ap=[[tpb2 * H, P], [H, half], [1, H]],
                            ),
                        )
                else:
                    nc.gpsimd.dma_start(
                        out=hch,
                        in_=h_flat[c * 2 * t:(c + 1) * 2 * t, :].rearrange(
                            "(p k) j -> p k j", k=tpb2),
                    )
                for k in range(tpb2):
                    vscr = scr_pool.tile([P, H], BF16, name="vscr", tag="vscr")
                    nc.vector.scalar_tensor_tensor(
                        out=vscr, in0=hch[:, k, :],
                        scalar=1.0, in1=q_pair_bf[:, c, :],
                        op0=mybir.AluOpType.mult,
                        op1=mybir.AluOpType.mult,
                        accum_out=s_all[:, k:k + 1],
                    )

                # ---- softmax statistics (per batch-half) ----
                m_col = stats_pool.tile([P, 1], F32, name="m_col", tag="m_col")
                nc.vector.reduce_max(m_col, s_all, axis=mybir.AxisListType.X)
                mT_ps = psum_pool.tile([1, P], F32, name="mT_ps", tag="mT")
                nc.tensor.transpose(mT_ps, m_col, ident)
                m01 = stats_pool.tile([1, 2], F32, name="m01", tag="m01")
                nc.vector.reduce_max(
                    m01[:, 0:1], mT_ps[:, 0:P // 2], axis=mybir.AxisListType.X
                )
                nc.vector.reduce_max(
                    m01[:, 1:2], mT_ps[:, P // 2:P], axis=mybir.AxisListType.X
                )
                mp_ps = psum_pool.tile([2, 1], F32, name="mp_ps", tag="mp")
                nc.tensor.transpose(mp_ps, m01, ident[0:1, 0:1])
                mp_sb = stats_pool.tile([2, 1], F32, name="mp_sb", tag="mp_sb")
                nc.vector.tensor_copy(mp_sb, mp_ps)
                negm_ps = psum_pool.tile([P, 1], F32, name="negm_ps", tag="negm")
                nc.tensor.matmul(
                    negm_ps, lhsT=negblk, rhs=mp_sb, start=True, stop=True
                )
                negm_sb = stats_pool.tile([P, 1], F32, name="negm_sb", tag="negm_sb")
                nc.vector.tensor_copy(negm_sb, negm_ps)
                p_all = stats_pool.tile([P, tpb2], BF16, name="p_all", tag="p_all")
                l_col = stats_pool.tile([P, 1], F32, name="l_col", tag="l_col")
                nc.scalar.activation(
                    out=p_all, in_=s_all, func=mybir.ActivationFunctionType.Exp,
                    bias=negm_sb, scale=1.0, accum_out=l_col,
                )
                l_ps = psum_pool.tile([2, 1], F32, name="l_ps", tag="l")
                nc.tensor.matmul(
                    l_ps, lhsT=blk, rhs=l_col, start=True, stop=True
                )
                inv_l = stats_pool.tile([2, 1], F32, name="inv_l", tag="inv_l")
                nc.vector.reciprocal(inv_l, l_ps)

                # ---- weighted sum for both halves in one matmul chain ----
                p_pair = stats_pool.tile(
                    [P, 2, tpb2], BF16, name="p_pair", tag="p_pair"
                )
                nc.vector.tensor_scalar(
                    p_pair[:, 0, :], p_all, mask_top, None,
                    op0=mybir.AluOpType.mult,
                )
                nc.vector.tensor_scalar(
                    p_pair[:, 1, :], p_all, mask_bot, None,
                    op0=mybir.AluOpType.mult,
                )
                ct_ps = psum_pool.tile([2, H], F32, name="ct_ps", tag="ct", bufs=2)
                for k in range(tpb2):
                    nc.tensor.matmul(
                        ct_ps, lhsT=p_pair[:, :, k],
                        rhs=hch[:, k, :],
                        start=(k == 0), stop=(k == tpb2 - 1),
                    )
                ct_sb = out_pool.tile([2, H], F32, name="ct_sb", tag="ct_sb")
                nc.vector.tensor_scalar(
                    ct_sb, ct_ps, inv_l, None, op0=mybir.AluOpType.mult
                )
                nc.sync.dma_start(out=out_d[2 * c:2 * c + 2, 0, :], in_=ct_sb)

    if not nc.is_finalized():
        nc.finalize()
    return nc


def build_nc(b_loc=B_LOC, t=T, chunk_k=64, h_bufs=5, reps=1,
             dual_ring=True, out_ring="sync", tri_ring=False,
             score_stride=1, bf16_chunks=True, out_batch=True,
             mix_chunks=False, mix_bufs=(3, 3), vscr_psum=False,
             q_psum=False):
    """Build the per-core Bass graph.

    chunk_k: T-tiles (of 128 timesteps) per DMA chunk.
    h_bufs:  chunk-pool slots (h_bufs * chunk_k * 128 timesteps resident).
    reps:    unrolled repetitions of the whole batch loop (benchmarking
             aid — wall(reps=3) - wall(reps=1) = 2x the kernel time,
             cancelling dispatch overhead).
    dual_ring: alternate chunk DMAs between the SP and ACT HWDGE rings.
    out_ring: 'sync' or 'scalar' — ring for the per-batch output DMA.
    out_batch: stage all b_loc ct rows in one partition-0 SBUF tile and
             write them with a single 8 KB DMA per rep instead of
             b_loc separate 1 KB DMAs.
    """
    tpb = t // P              # T-tiles per batch
    n_chunks = tpb // chunk_k
    hc = H // P               # contraction chunks for the q matmul

    from concourse.bacc import Bacc

    nc = Bacc()
    h_d = nc.declare_dram_parameter("h_0_t", [b_loc, t, H], F32, isOutput=False)
    ht_d = nc.declare_dram_parameter("ht", [b_loc, 1, H], F32, isOutput=False)
    w_d = nc.declare_dram_parameter("weight", [H, H], F32, isOutput=False)
    out_d = nc.declare_dram_parameter("out", [b_loc, 1, H], F32, isOutput=True)

    with TileContext(nc) as tc:
        with (
            tc.tile_pool(name="const", bufs=1) as const_pool,
            tc.tile_pool(name="hbuf", bufs=h_bufs) as h_pool,
            tc.tile_pool(name="stats", bufs=2) as stats_pool,
            tc.tile_pool(name="scr", bufs=2) as scr_pool,
            tc.tile_pool(name="ctout", bufs=2) as out_pool,
            tc.tile_pool(name="ps", bufs=1, space="PSUM") as psum_pool,
            tc.tile_pool(name="qdram", bufs=1, space="DRAM") as dram_pool,
        ):
            # ---- constants ----
            ones_col = const_pool.tile([P, 1], F32, name="ones_col")
            nc.vector.memset(ones_col, 1.0)
            neg_ones_row = const_pool.tile([1, P], F32, name="neg_ones_row")
            nc.vector.memset(neg_ones_row, -1.0)
            ident_i = const_pool.tile([P, P], I32, name="ident_i")
            nc.gpsimd.iota(ident_i, pattern=[[-1, P]], base=0, channel_multiplier=1)
            ident = const_pool.tile([P, P], F32, name="ident")
            nc.vector.tensor_scalar(
                ident, ident_i, 0, None, op0=mybir.AluOpType.is_equal
            )

            # ---- q = ht @ W for all local batches (one-time setup) ----
            w_sb = const_pool.tile([P, hc, H], F32, name="w_sb")
            nc.sync.dma_start(
                out=w_sb, in_=w_d[:].rearrange("(c p) k -> p c k", p=P)
            )
            htT = const_pool.tile([P, hc, b_loc], F32, name="htT")
            for c in range(hc):
                nc.gpsimd.dma_start(
                    out=htT[:, c, :],
                    in_=ht_d[:, 0, c * P : (c + 1) * P].rearrange("b p -> p b"),
                )
            # dummy self-matmul absorbs the htT DMA wait so the q matmul
            # carries a single sync wait (PE LDWEIGHTS allows only one)
            dmy_ps = psum_pool.tile(
                [b_loc, b_loc], F32, name="dmy_ps", tag="dmy",
                bufs=1 if (vscr_psum or q_psum) else 2,
            )
            nc.tensor.matmul(
                dmy_ps, lhsT=htT[:, 0, :], rhs=htT[:, 0, :], start=True, stop=True
            )
            q_ps = psum_pool.tile([b_loc, H], F32, name="q_ps", tag="qps")
            for c in range(hc):
                nc.tensor.matmul(
                    q_ps, lhsT=htT[:, c, :], rhs=w_sb[:, c, :],
                    start=(c == 0), stop=(c == hc - 1),
                )
            q_sb = const_pool.tile([b_loc, H], F32, name="q_sb")
            nc.vector.tensor_copy(q_sb, q_ps)
            # replicate each batch's q across all 128 partitions (DRAM bounce)
            q_dram = dram_pool.tile([b_loc, H], F32, name="q_dram")
            nc.sync.dma_start(out=q_dram, in_=q_sb)
            q_rep = const_pool.tile([P, b_loc, H], F32, name="q_rep")
            q_bcast_src = bass.AP(
                tensor=q_dram.tensor, offset=q_dram.offset,
                ap=[[0, P], [H, b_loc], [1, H]],
            )
            nc.sync.dma_start(out=q_rep, in_=q_bcast_src)
            if bf16_chunks:
                BF16 = mybir.dt.float16
                if q_psum:
                    # DOES NOT COMPILE: TRN2 DVE cannot write 16-bit data
                    # to PSUM (Cast fails tensor4d_valid), so the per-
                    # score-inst q reads cannot be moved off SBUF; kept
                    # for reference with vscr_psum (same ISA family)
                    q_rep_pa = psum_pool.tile(
                        [P, b_loc // 2, H], BF16, name="q_rep_pa", tag="qra"
                    )
                    q_rep_pb = psum_pool.tile(
                        [P, b_loc // 2, H], BF16, name="q_rep_pb", tag="qrb"
                    )
                    nc.vector.tensor_copy(q_rep_pa, q_rep[:, :b_loc // 2, :])
                    nc.vector.tensor_copy(q_rep_pb, q_rep[:, b_loc // 2:, :])

                    def q_bf(b):
                        if b < b_loc // 2:
                            return q_rep_pa[:, b, :]
                        return q_rep_pb[:, b - b_loc // 2, :]
                else:
                    q_rep_bf = const_pool.tile(
                        [P, b_loc, H], BF16, name="q_rep_bf"
                    )
                    nc.vector.tensor_copy(q_rep_bf, q_rep)

                    def q_bf(b):
                        return q_rep_bf[:, b, :]

            # ---- batch loop ----
            ct_all = None
            for b in [bb for _ in range(reps) for bb in range(b_loc)]:
                if out_batch and b == 0:
                    ct_all = out_pool.tile(
                        [1, b_loc, H], F32, name="ct_all", tag="ct_all"
                    )
                s_all = stats_pool.tile([P, tpb], F32, name="s_all", tag="s_all")
                if mix_chunks:
                    # Half the batch as fp16 via SWDGE (gpsimd, cast in the
                    # SDMA datapath), half as raw f32r via HWDGE
                    # (sync/scalar alternating): two DMA queues pull
                    # concurrently, which wins slightly more HBM arbitration
                    # share when the stack is contended.
                    th = tpb // 2
                    BF16 = mybir.dt.float16
                    hch_a = h_pool.tile(
                        [P, th, H], BF16, name="hch_a", tag="hcha",
                        bufs=mix_bufs[0],
                    )
                    nc.gpsimd.dma_start(
                        out=hch_a,
                        in_=h_d[b, 0:t // 2, :].rearrange(
                            "(p k) j -> p k j", k=th),
                    )
                    hch_b = h_pool.tile(
                        [P, th, H], F32R, name="hch_b", tag="hchb",
                        bufs=mix_bufs[1],
                    )
                    eng = nc.scalar if b % 2 else nc.sync
                    eng.dma_start(
                        out=hch_b,
                        in_=h_d[b, t // 2:, :].rearrange(
                            "(p k) j -> p k j", k=th).bitcast(F32R),
                    )
                    for k in range(th):
                        vscr = scr_pool.tile(
                            [P, H], BF16, name="vscr", tag="vscr"
                        )
                        nc.vector.scalar_tensor_tensor(
                            out=vscr, in0=hch_a[:, k, :],
                            scalar=1.0, in1=q_bf(b),
                            op0=mybir.AluOpType.mult,
                            op1=mybir.AluOpType.mult,
                            accum_out=s_all[:, k:k + 1],
                        )
                    for k in range(th):
                        vscrf = scr_pool.tile(
                            [P, H], F32, name="vscrf", tag="vscrf"
                        )
                        nc.vector.scalar_tensor_tensor(
                            out=vscrf, in0=hch_b[:, k, :].bitcast(F32),
                            scalar=1.0, in1=q_rep[:, b, :],
                            op0=mybir.AluOpType.mult,
                            op1=mybir.AluOpType.mult,
                            accum_out=s_all[:, th + k:th + k + 1],
                        )
                    chunks = [(hch_a, hch_b)]
                else:
                    chunks = []
                for c in range(n_chunks) if not mix_chunks else []:
                    # declared fp32r so TensorE can consume it at full rate;
                    # the DVE scores path reads the same bits as fp32.
                    # Blocked T layout: partition p holds chunk_k CONSECUTIVE
                    # timesteps (contiguous chunk_k*1KB DRAM per partition ->
                    # large DMA descriptors). softmax + weighted sum are
                    # permutation-invariant over T, so the order change is
                    # harmless.
                    src = h_d[b, c * chunk_k * P : (c + 1) * chunk_k * P, :]
                    if bf16_chunks:
                        # SWDGE casts f32->fp16 inside the SDMA datapath
                        # (free); HBM read side is unchanged, SBUF write
                        # side halves, DVE scores run at 2x on 16-bit and
                        # PE consumes bf16 at full rate.
                        hch = h_pool.tile(
                            [P, chunk_k, H], mybir.dt.float16,
                            name="hch", tag="hch",
                        )
                        nc.gpsimd.dma_start(
                            out=hch,
                            in_=src.rearrange("(p k) j -> p k j", k=chunk_k),
                        )
                    else:
                        hch = h_pool.tile(
                            [P, chunk_k, H], F32R, name="hch", tag="hch"
                        )
                        if tri_ring:
                            dma_eng = [nc.sync, nc.scalar, nc.gpsimd][c % 3]
                        else:
                            dma_eng = (
                                nc.scalar if (dual_ring and c % 2) else nc.sync
                            )
                        dma_eng.dma_start(
                            out=hch,
                            in_=src.rearrange(
                                "(p k) j -> p k j", k=chunk_k
                            ).bitcast(F32R),
                        )
                    chunks.append(hch)
                    for k in range(0, chunk_k, score_stride):
                        gk = c * chunk_k + k
                        if bf16_chunks:
                            if vscr_psum:
                                # DOES NOT COMPILE: the ISA rejects a PSUM
                                # main out for ScalarTensorTensor
                                # (tensor2d_valid); the dead elementwise-
                                # product write cannot leave SBUF
                                vscr = psum_pool.tile(
                                    [P, H], mybir.dt.float16, name="vscr",
                                    tag="vscr", bufs=1,
                                )
                            else:
                                vscr = scr_pool.tile(
                                    [P, H], mybir.dt.float16, name="vscr",
                                    tag="vscr",
                                )
                            nc.vector.scalar_tensor_tensor(
                                out=vscr, in0=hch[:, k, :],
                                scalar=1.0, in1=q_bf(b),
                                op0=mybir.AluOpType.mult,
                                op1=mybir.AluOpType.mult,
                                accum_out=s_all[:, gk : gk + 1],
                            )
                        else:
                            vscr = scr_pool.tile(
                                [P, H], F32, name="vscr", tag="vscr"
                            )
                            nc.vector.scalar_tensor_tensor(
                                out=vscr, in0=hch[:, k, :].bitcast(F32),
                                scalar=1.0, in1=q_rep[:, b, :],
                                op0=mybir.AluOpType.mult,
                                op1=mybir.AluOpType.mult,
                                accum_out=s_all[:, gk : gk + 1],
                            )

                # ---- softmax statistics ----
                m_col = stats_pool.tile([P, 1], F32, name="m_col", tag="m_col")
                nc.vector.reduce_max(m_col, s_all, axis=mybir.AxisListType.X)
                mT_ps = psum_pool.tile([1, P], F32, name="mT_ps", tag="mT")
                nc.tensor.transpose(mT_ps, m_col, ident)
                m_sb = stats_pool.tile([1, 1], F32, name="m_sb", tag="m_sb")
                nc.vector.reduce_max(m_sb, mT_ps, axis=mybir.AxisListType.X)
                # broadcast -max to all partitions via a C=1 matmul
                negm_ps = psum_pool.tile([P, 1], F32, name="negm_ps", tag="negm")
                nc.tensor.matmul(
                    negm_ps, lhsT=neg_ones_row, rhs=m_sb, start=True, stop=True
                )
                negm_sb = stats_pool.tile([P, 1], F32, name="negm_sb", tag="negm_sb")
                nc.vector.tensor_copy(negm_sb, negm_ps)
                p_dt = mybir.dt.float16 if bf16_chunks else F32R
                p_all = stats_pool.tile([P, tpb], p_dt, name="p_all", tag="p_all")
                l_col = stats_pool.tile([P, 1], F32, name="l_col", tag="l_col")
                nc.scalar.activation(
                    out=p_all, in_=s_all, func=mybir.ActivationFunctionType.Exp,
                    bias=negm_sb, scale=1.0, accum_out=l_col,
                )
                l_ps = psum_pool.tile([1, 1], F32, name="l_ps", tag="l")
                nc.tensor.matmul(
                    l_ps, lhsT=l_col, rhs=ones_col, start=True, stop=True
                )
                inv_l = stats_pool.tile([1, 1], F32, name="inv_l", tag="inv_l")
                nc.vector.reciprocal(inv_l, l_ps)

                # ---- weighted sum over T on TensorE ----
                ct_ps = psum_pool.tile(
                    [1, H], F32, name="ct_ps", tag="ct",
                    bufs=1 if q_psum else 2,
                )
                if mix_chunks:
                    th = tpb // 2
                    hch_a, hch_b = chunks[0]
                    # f32r copy of the f32-half weights so lhsT/rhs dtypes
                    # match within each matmul
                    p_r = stats_pool.tile([P, th], F32R, name="p_r", tag="p_r")
                    nc.vector.tensor_copy(p_r, p_all[:, th:tpb])
                    for k in range(th):
                        nc.tensor.matmul(
                            ct_ps, lhsT=p_all[:, k:k + 1],
                            rhs=hch_a[:, k, :],
                            start=(k == 0), stop=False,
                        )
                    for k in range(th):
                        nc.tensor.matmul(
                            ct_ps, lhsT=p_r[:, k:k + 1],
                            rhs=hch_b[:, k, :],
                            start=False, stop=(k == th - 1),
                        )
                else:
                    for c in range(n_chunks):
                        for k in range(chunk_k):
                            gk = c * chunk_k + k
                            nc.tensor.matmul(
                                ct_ps, lhsT=p_all[:, gk : gk + 1],
                                rhs=chunks[c][:, k, :],
                                start=(gk == 0), stop=(gk == tpb - 1),
                            )
                out_eng = nc.scalar if out_ring == "scalar" else nc.sync
                if out_batch:
                    nc.vector.tensor_scalar_mul(
                        ct_all[:, b, :], ct_ps, inv_l[0:1, 0:1]
                    )
                    if b == b_loc - 1:
                        out_eng.dma_start(
                            out=out_d[:].rearrange("b o h -> o b h"),
                            in_=ct_all,
                        )
                else:
                    ct_sb = out_pool.tile([1, H], F32, name="ct_sb", tag="ct_sb")
                    nc.vector.tensor_scalar_mul(ct_sb, ct_ps, inv_l[0:1, 0:1])
                    out_eng.dma_start(out=out_d[b, :, :], in_=ct_sb)

    # Bacc.finalize() runs the lowering passes raw Bass lacks: matmul-wait
    # relocation, event-semaphore wait splitting (HW allows 1 wait/inst),
    # GPSIMD library loads, ACT table loads, and extended-ISA codegen.
    if not nc.is_finalized():
        nc.finalize()
    return nc


_nc_cache = None


def _get_nc():
    global _nc_cache
    if _nc_cache is None:
        _nc_cache = build_nc()
    return _nc_cache


def _run(inputs, trace=False, nc=None, **kw):
    if nc is None:
        nc = _get_nc()
    ht = np.ascontiguousarray(np.asarray(inputs["ht"], dtype=np.float32))
    h0 = np.asarray(inputs["h_0_t"], dtype=np.float32)
    w = np.ascontiguousarray(np.asarray(inputs["weight"], dtype=np.float32))
    in_maps = []
    for i in range(N_CORES):
        sl = slice(i * B_LOC, (i + 1) * B_LOC)
        in_maps.append(
            {
                "h_0_t": np.ascontiguousarray(h0[sl]),
                "ht": np.ascontiguousarray(ht[sl]),
                "weight": w,
            }
        )
    res = run_bass_kernel_spmd(
        nc, in_maps, core_ids=list(range(N_CORES)), trace=trace, **kw
    )
    out = np.concatenate([r["out"] for r in res.results], axis=0)
    return out, res


def kernel(**inputs):
    out, _ = _run(inputs)
    return out


# ---------------------------------------------------------------------------
# Timing helper (used by test.py only; not part of the grading contract).
# Rebuilds the shard_map executable once so repeat calls reuse one compiled
# NEFF with device-resident inputs, then reports min wall-clock.
# ---------------------------------------------------------------------------


_nc_rep_cache = {}


def _get_exec(inputs, reps=1):
    """Build (once) and return a zero-arg callable running the reps-unrolled
    kernel on all 8 cores with device-resident inputs."""
    import jax
    from jax.experimental.shard_map import shard_map
    from jax.sharding import Mesh, NamedSharding, PartitionSpec

    from concourse import bass2jax

    if reps == 1:
        nc = _get_nc()
    else:
        if reps not in _nc_rep_cache:
            _nc_rep_cache[reps] = build_nc(reps=reps)
        nc = _nc_rep_cache[reps]
    bass2jax.install_neuronx_cc_hook()

    partition_name = (
        nc.partition_id_tensor.name if nc.partition_id_tensor else None
    )
    in_names, out_names, out_avals, zero_outs = [], [], [], []
    for alloc in nc.m.functions[0].allocations:
        if not isinstance(alloc, mybir.MemoryLocationSet):
            continue
        name = alloc.memorylocations[0].name
        if alloc.kind == "ExternalInput":
            if name != partition_name:
                in_names.append(name)
        elif alloc.kind == "ExternalOutput":
            out_names.append(name)
            shape = tuple(alloc.tensor_shape)
            dtype = mybir.dt.np(alloc.dtype)
            out_avals.append(jax.core.ShapedArray(shape, dtype))
            zero_outs.append(np.zeros(shape, dtype))
    n_params = len(in_names)
    n_outs = len(out_avals)
    all_names = list(in_names) + out_names
    if partition_name is not None:
        all_names.append(partition_name)

    def _body(*args):
        operands = list(args)
        if partition_name is not None:
            operands.append(bass2jax.partition_id_tensor())
        outs = bass2jax._bass_exec_p.bind(
            *operands,
            out_avals=tuple(out_avals),
            in_names=tuple(all_names),
            out_names=tuple(out_names),
            lowering_input_output_aliases=(),
            sim_require_finite=True,
            sim_require_nnan=True,
            nc=nc,
        )
        return tuple(outs)

    devices = jax.devices()[:N_CORES]
    mesh = Mesh(np.asarray(devices), ("core",))
    in_specs = (PartitionSpec("core"),) * (n_params + n_outs)
    out_specs = (PartitionSpec("core"),) * n_outs
    sharded = jax.jit(
        shard_map(
            _body, mesh=mesh, in_specs=in_specs, out_specs=out_specs,
            check_rep=False,
        ),
        keep_unused=True,
    )

    ht = np.ascontiguousarray(np.asarray(inputs["ht"], dtype=np.float32))
    h0 = np.ascontiguousarray(np.asarray(inputs["h_0_t"], dtype=np.float32))
    w = np.asarray(inputs["weight"], dtype=np.float32)
    per_core = {
        "h_0_t": h0,
        "ht": ht,
        "weight": np.concatenate([w[None]] * N_CORES, axis=0).reshape(
            N_CORES * w.shape[0], w.shape[1]
        ),
    }
    sh = NamedSharding(mesh, PartitionSpec("core"))
    xs = [jax.device_put(per_core[name], sh) for name in in_names]
    zs = [
        jax.device_put(
            np.zeros((N_CORES * z.shape[0], *z.shape[1:]), z.dtype), sh
        )
        for z in zero_outs
    ]

    def call():
        jax.block_until_ready(sharded(*xs, *zs))

    call()  # warm up (includes compile)
    return call


def time_kernel_pair(inputs, iters=60, reps_hi=3, reps_lo=1):
    """Interleaved slope timing: min(wall_hi) - min(wall_lo) over paired
    adjacent samples cancels axon dispatch overhead and its drift.
    Returns one kernel execution time in ns."""
    import time

    lo = _get_exec(inputs, reps=reps_lo)
    hi = _get_exec(inputs, reps=reps_hi)
    t_lo, t_hi = [], []
    for _ in range(iters):
        t0 = time.perf_counter()
        lo()
        t1 = time.perf_counter()
        hi()
        t2 = time.perf_counter()
        t_lo.append(t1 - t0)
        t_hi.append(t2 - t1)
    ns = (min(t_hi) - min(t_lo)) / (reps_hi - reps_lo) * 1e9
    return ns, min(t_lo) * 1e9, min(t_hi) * 1e9

